# revision 1
# baseline (speedup 1.0000x reference)
"""TRN2 Bass kernel for the 4-layer encoder-with-reaches model
(nn_EncoderPreTre: B=8, S=512, D=1024, H=16 heads, NL=4 layers).

kernel(**inputs) takes the FULL inputs (src, reaches, emb_table,
qw/kw/vw/ow) and returns the full output tuple (emb, x) matching
reference.reference(). Distribution: data-parallel over the batch —
core b computes batch element b end to end (B == 8 == n_cores); the
embedding-row gather and per-batch contrib vectors are the host-side
sharding step.

Numerics: this model amplifies matmul rounding ~1000x (logits reach
5.6e6; contrib scaling grows x ~40x per layer), so bf16/tf32-class
matmuls fail. Everything runs at fp32 fidelity:
  - scores / P-transposes / attention-value / output projections are
    native fp32 matmuls (4 cycles/row on the PE);
  - the q/k/v/OV projections use a 3-term float32r hi/lo split
    (Wh@xh + Wh@xl + Wl@xh, each 1 cycle/row) with weights pre-split on
    the host into tf32-representable halves and the residual stream kept
    as a float32r (hi, lo) pair — fp32-equivalent accuracy at ~2.6x the
    fp32 matmul throughput.

Per-core dataflow (residual transposed: x^T [1024, 512] in SBUF):
  P1: q^T = (qw/8)^T-proj, k^T, v in [s,do] layout with v' = v*reaches
      fused into the PSUM->SBUF copy.
  P2 per head: scores[q,k] -> row-max (DVE reduce, negated) ->
      E = exp(s-m) with row-sum Z from the same ACT op (accum_out; the
      softmax numerator/denominator use the same PSUM values, keeping it
      consistent at huge logit scale) -> P = (E*(-c/Z)[q])*diagmask in
      one scalar_tensor_tensor -> P transposed 128x128-blockwise via PE
      transpose-mode -> M2T[dk,q] = sum_k v'[k,dk]*PT[k,q].
  P3: x += (OV@x)*c + ow-proj(concatT), OV = ow@vw folded on the host so
      the reference's "v - P@v'" becomes two accumulating projections.

Implementation notes:
  - Residual stored as f32r pair (xhi, xlo): x ~= xhi + xlo to ~2^-22.
  - Weights for q/k/v/OV pre-split on host into tf32-representable hi/lo
    parts, DMA'd as float32r tiles (verifier accepts f32r-typed producers).
  - Projections computed as 3-term splits: Wh@xh + Wh@xl + Wl@xh, each a
    1-cycle/row f32r matmul (vs 4 cycles/row for fp32).
  - scores / transposes / M2 / OW2 remain fp32 (their operands are
    device-produced fp32 tensors; splitting them costs more DVE than it
    saves PE).
"""
import numpy as np

import concourse.tile as tile
from concourse import bacc, mybir
from concourse.bass_utils import run_bass_kernel_spmd

F32 = mybir.dt.float32
F32R = mybir.dt.float32r
BF16 = mybir.dt.bfloat16
AX = mybir.AxisListType
OP = mybir.AluOpType
AF = mybir.ActivationFunctionType

B, S, D, H, DK, NL = 8, 512, 1024, 16, 64, 4
QC = S // 128
KC = S // 128
DC = D // 128


TRACE = False        # test harness sets True for neuron-profile capture
LAST_RESULT = None   # BassKernelResults of the last kernel() call
_NC_CACHE = {}


def _build(n_layers=NL, n_cores=8):
    nc = bacc.Bacc("TRN2", target_bir_lowering=False, debug=False,
                   num_devices=n_cores)
    d_x0 = nc.dram_tensor("x0t", [D, S], F32, kind="ExternalInput").ap()
    dw = {}
    for nm in ["wq", "wk", "wv", "wov"]:
        dw[nm + "h"] = nc.dram_tensor(nm + "h", [NL, D, D], F32R,
                                      kind="ExternalInput").ap()
        dw[nm + "l"] = nc.dram_tensor(nm + "l", [NL, D, D], F32R,
                                      kind="ExternalInput").ap()
    dw["wo"] = nc.dram_tensor("wo", [NL, D, D], F32, kind="ExternalInput").ap()
    d_cb = nc.dram_tensor("cb", [128, S], F32, kind="ExternalInput").ap()
    d_negc = nc.dram_tensor("negc", [128, QC], F32, kind="ExternalInput").ap()
    d_rr = nc.dram_tensor("rr", [128, KC], F32, kind="ExternalInput").ap()
    d_mask = nc.dram_tensor("maskq", [QC, 128, S], mybir.dt.bfloat16, kind="ExternalInput").ap()
    d_id = nc.dram_tensor("ident", [128, 128], F32, kind="ExternalInput").ap()
    d_out = nc.dram_tensor("xt", [D, S], F32, kind="ExternalOutput").ap()

    with tile.TileContext(nc) as tc:
        _emit(nc, tc, n_layers, d_x0, dw,
              d_cb, d_negc, d_rr, d_mask, d_id, d_out)
    nc.compile()
    return nc


def _emit(nc, tc, n_layers, d_x0, dw, d_cb, d_negc, d_rr, d_mask, d_id, d_out):
    ctx_pools = []

    def pool(name, bufs, space="SBUF"):
        p = tc.tile_pool(name=name, bufs=bufs, space=space)
        ctx_pools.append(p)
        return p.__enter__()

    const = pool("const", 1)
    xpool = pool("x", 1)
    actp = pool("act", 1)
    wpool = pool("w", 1)          # 16 tags (hi/lo per ki); ki-granular overlap
    epool = pool("E", 2)
    ppool = pool("P", 4)
    ptpool = pool("PT", 4)
    small = pool("small", 3)
    tmp8 = pool("tmp8", 1)
    psA = pool("psA", 3, "PSUM")
    psB = pool("psB", 2, "PSUM")
    psC = pool("psC", 2, "PSUM")

    cb = const.tile([128, S], F32)
    nc.sync.dma_start(cb[:], d_cb)
    negc = const.tile([128, QC], F32)
    nc.sync.dma_start(negc[:], d_negc)
    rr = const.tile([128, KC], F32)
    nc.sync.dma_start(rr[:], d_rr)
    ident = const.tile([128, 128], F32)
    nc.sync.dma_start(ident[:], d_id)
    masks = []
    for t in range(QC):
        mt = const.tile([128, S], BF16, tag=f"mask{t}", name=f"mask{t}")
        nc.sync.dma_start(mt[:], d_mask[t])
        masks.append(mt)

    # residual pair: x ~= xhi + xlo (f32r each)
    xhi, xlo = [], []
    for c in range(DC):
        xh = xpool.tile([128, S], F32R, tag=f"xh{c}", name=f"xh{c}")
        xl = xpool.tile([128, S], F32R, tag=f"xl{c}", name=f"xl{c}")
        xhi.append(xh)
        xlo.append(xl)
    for c in range(DC):
        xf = tmp8.tile([128, S], F32, tag="xn", name=f"x0f{c}", bufs=1)
        nc.sync.dma_start(xf[:], d_x0[c * 128:(c + 1) * 128, :])
        nc.vector.tensor_copy(xhi[c][:], xf[:])
        nc.vector.tensor_tensor(xlo[c][:], xf[:], xhi[c][:], op=OP.subtract)

    for l in range(n_layers):
        def load_split(nm):
            his, los = [], []
            for ki in range(DC):
                wh = wpool.tile([128, D], F32R, tag=f"w{ki}h", name=f"{nm}h{ki}_{l}")
                nc.sync.dma_start(wh[:], dw[nm + "h"][l, ki * 128:(ki + 1) * 128, :])
                wl = wpool.tile([128, D], F32R, tag=f"w{ki}l", name=f"{nm}l{ki}_{l}")
                nc.sync.dma_start(wl[:], dw[nm + "l"][l, ki * 128:(ki + 1) * 128, :])
                his.append(wh)
                los.append(wl)
            return his, los

        def proj_split(his, los, outtag, rhs_hi, rhs_lo):
            outs = []
            for c in range(DC):
                p = psA.tile([128, S], F32, tag="psA", name=f"pp{outtag}{c}_{l}")
                n_mm = 3 * DC
                i_mm = 0
                sl = slice(c * 128, (c + 1) * 128)
                for ki in range(DC):
                    for lhsT, rhs in ((his[ki][:, sl], rhs_hi[ki][:]),
                                      (his[ki][:, sl], rhs_lo[ki][:]),
                                      (los[ki][:, sl], rhs_hi[ki][:])):
                        nc.tensor.matmul(
                            p[:], lhsT, rhs, start=(i_mm == 0),
                            stop=(i_mm == n_mm - 1), skip_group_check=True)
                        i_mm += 1
                o = actp.tile([128, S], F32, tag=f"{outtag}{c}",
                              name=f"{outtag}{c}_{l}")
                nc.vector.tensor_copy(o[:], p[:])
                outs.append(o)
            return outs

        qh, ql = load_split("wq")
        qt = proj_split(qh, ql, "qt", xhi, xlo)
        kh, kl = load_split("wk")
        kt = proj_split(kh, kl, "kt", xhi, xlo)

        vh, vl = load_split("wv")
        vp = []
        for sc in range(KC):
            vtile = actp.tile([128, D], F32, tag=f"vp{sc}", name=f"vp{sc}_{l}")
            ssl = slice(sc * 128, (sc + 1) * 128)
            for half in range(2):
                hsl = slice(half * 512, (half + 1) * 512)
                p = psA.tile([128, S], F32, tag="psA", name=f"pv{sc}{half}_{l}")
                n_mm = 3 * DC
                i_mm = 0
                for ki in range(DC):
                    for lhsT, rhs in ((xhi[ki][:, ssl], vh[ki][:, hsl]),
                                      (xlo[ki][:, ssl], vh[ki][:, hsl]),
                                      (xhi[ki][:, ssl], vl[ki][:, hsl])):
                        nc.tensor.matmul(p[:], lhsT, rhs, start=(i_mm == 0),
                                         stop=(i_mm == n_mm - 1),
                                         skip_group_check=True)
                        i_mm += 1
                nc.vector.tensor_scalar(
                    vtile[:, hsl], p[:], rr[:, sc:sc + 1], None, op0=OP.mult)
            vp.append(vtile)

        concatT = [actp.tile([128, S], F32, tag=f"cc{c}", name=f"cc{c}_{l}")
                   for c in range(DC)]
        for h in range(H):
            hp = h // 2
            hb = (h % 2) * 64
            qsl = qt[hp][hb:hb + 64, :]
            ksl = kt[hp][hb:hb + 64, :]

            negm = small.tile([128, QC], F32, tag="negm", name=f"negm{h}_{l}")
            zst = small.tile([128, QC], F32, tag="zst", name=f"zst{h}_{l}")
            sc_t = small.tile([128, QC], F32, tag="scl", name=f"scl{h}_{l}")
            Ps = []
            for t in range(QC):
                ps = psA.tile([128, S], F32, tag="psA", name=f"sc{h}{t}_{l}")
                nc.tensor.matmul(ps[:], qsl[:, t * 128:(t + 1) * 128], ksl,
                                 start=True, stop=True)
                nc.vector.tensor_reduce(
                    negm[:, t:t + 1], ps[:], axis=AX.X, op=OP.max, negate=True)
                e = epool.tile([128, S], F32, tag="E", name=f"e{h}{t}_{l}")
                nc.scalar.activation(e[:], ps[:], AF.Exp,
                                     bias=negm[:, t:t + 1], scale=1.0,
                                     accum_out=zst[:, t:t + 1])
                nc.vector.reciprocal(sc_t[:, t:t + 1], zst[:, t:t + 1])
                nc.vector.tensor_tensor(
                    sc_t[:, t:t + 1], sc_t[:, t:t + 1], negc[:, t:t + 1],
                    op=OP.mult)
                p = ppool.tile([128, S], F32, tag="P", name=f"p{h}{t}_{l}")
                nc.vector.scalar_tensor_tensor(
                    p[:], e[:], sc_t[:, t:t + 1], masks[t][:],
                    op0=OP.mult, op1=OP.mult)
                Ps.append(p)

            PTs = []
            for kc in range(KC):
                tp = psB.tile([128, S], F32, tag="psB", name=f"tp{h}{kc}_{l}")
                for t in range(QC):
                    nc.tensor.matmul(
                        tp[:, t * 128:(t + 1) * 128],
                        Ps[t][:, kc * 128:(kc + 1) * 128], ident[:],
                        is_transpose=True, start=(t == 0), stop=(t == QC - 1),
                        skip_group_check=True)
                pt_sb = ptpool.tile([128, S], F32, tag="PT", name=f"pt{h}{kc}_{l}")
                if kc % 2 == 0:
                    nc.vector.tensor_copy(pt_sb[:], tp[:])
                else:
                    nc.scalar.copy(pt_sb[:], tp[:])
                PTs.append(pt_sb)

            m2 = psC.tile([128, S], F32, tag="psC", name=f"m2{h}_{l}")
            off = hb
            for kc in range(KC):
                nc.tensor.matmul(
                    m2[off:off + 64, :], vp[kc][:, h * 64:h * 64 + 64],
                    PTs[kc][:], start=(kc == 0), stop=(kc == KC - 1))
            nc.vector.tensor_copy(concatT[hp][hb:hb + 64, :], m2[off:off + 64, :])

        ovh, ovl = load_split("wov")
        wo_t = []
        for ki in range(DC):
            wt = wpool.tile([128, D], F32, tag=f"w{ki}h", name=f"wo{ki}_{l}")
            nc.sync.dma_start(wt[:], dw["wo"][l, ki * 128:(ki + 1) * 128, :])
            wo_t.append(wt)

        t1s = []
        for c in range(DC):
            pov = psC.tile([128, S], F32, tag="psC", name=f"pov{c}_{l}")
            n_mm = 3 * DC
            i_mm = 0
            sl = slice(c * 128, (c + 1) * 128)
            for ki in range(DC):
                for lhsT, rhs in ((ovh[ki][:, sl], xhi[ki][:]),
                                  (ovh[ki][:, sl], xlo[ki][:]),
                                  (ovl[ki][:, sl], xhi[ki][:])):
                    nc.tensor.matmul(pov[:], lhsT, rhs, start=(i_mm == 0),
                                     stop=(i_mm == n_mm - 1),
                                     skip_group_check=True)
                    i_mm += 1
            t1 = tmp8.tile([128, S], F32, tag=f"t1{c}", name=f"t1{c}_{l}", bufs=1)
            nc.vector.tensor_tensor(t1[:], pov[:], cb[:], op=OP.mult)
            t1s.append(t1)
        for c in range(DC):
            pow_ = psB.tile([128, S], F32, tag="psB", name=f"pow{c}_{l}")
            for ki in range(DC):
                nc.tensor.matmul(
                    pow_[:], wo_t[ki][:, c * 128:(c + 1) * 128], concatT[ki][:],
                    start=(ki == 0), stop=(ki == DC - 1))
            # rebuild full-precision x, add delta, re-split into the pair
            xt_new = tmp8.tile([128, S], F32, tag="xn", name=f"xn{c}_{l}", bufs=1)
            nc.vector.tensor_tensor(xt_new[:], xhi[c][:], xlo[c][:], op=OP.add)
            nc.vector.tensor_tensor(xt_new[:], xt_new[:], pow_[:], op=OP.add)
            nc.vector.tensor_tensor(xt_new[:], xt_new[:], t1s[c][:], op=OP.add)
            if l == n_layers - 1:
                nc.sync.dma_start(d_out[c * 128:(c + 1) * 128, :], xt_new[:])
            else:
                nc.vector.tensor_copy(xhi[c][:], xt_new[:])
                nc.vector.tensor_tensor(xlo[c][:], xt_new[:], xhi[c][:],
                                        op=OP.subtract)

    for p in reversed(ctx_pools):
        p.__exit__(None, None, None)


# ---------------- host side ----------------

def _t13(a):
    u = np.ascontiguousarray(np.asarray(a, np.float32)).view(np.uint32)
    r = (u + np.uint32(1 << 12)) & ~np.uint32((1 << 13) - 1)
    return r.view(np.float32)


def _host_prep(src, reaches, emb_table, qw, kw, vw, ow):
    src = np.asarray(src)
    reaches = np.asarray(reaches, dtype=np.float32)
    emb_table = np.asarray(emb_table, dtype=np.float32)
    emb = emb_table[src]
    rs = reaches.sum(-1, keepdims=True)
    contrib = ((rs - reaches) / (rs + 1e-9) * (1.0 - reaches) * 100.0
               ).astype(np.float32)

    qw = np.asarray(qw, np.float32); kw = np.asarray(kw, np.float32)
    vw = np.asarray(vw, np.float32); ow = np.asarray(ow, np.float32)
    wq = np.ascontiguousarray(np.transpose(qw, (0, 2, 1)) * 0.125)
    wk = np.ascontiguousarray(np.transpose(kw, (0, 2, 1)))
    wv = np.ascontiguousarray(np.transpose(vw, (0, 2, 1)))
    wo = np.ascontiguousarray(np.transpose(ow, (0, 2, 1)))
    wov = np.stack([
        np.ascontiguousarray(
            (ow[l].astype(np.float64) @ vw[l].astype(np.float64)).T
        ).astype(np.float32)
        for l in range(NL)])

    def split(w):
        h = _t13(w)
        lo = _t13(w - h)
        return np.ascontiguousarray(h), np.ascontiguousarray(lo)

    wqh, wql = split(wq)
    wkh, wkl = split(wk)
    wvh, wvl = split(wv)
    wovh, wovl = split(wov)

    import ml_dtypes
    maskq = np.ones((QC, 128, S), ml_dtypes.bfloat16)
    idx = np.arange(128)
    diagval = np.float32(1.0) - np.float32(0.999999)
    for t in range(QC):
        maskq[t, idx, t * 128 + idx] = ml_dtypes.bfloat16(diagval)
    ident = np.eye(128, dtype=np.float32)

    shared = dict(wqh=wqh, wql=wql, wkh=wkh, wkl=wkl, wvh=wvh, wvl=wvl,
                  wovh=wovh, wovl=wovl, wo=wo, maskq=maskq, ident=ident)
    in_maps = []
    for b in range(B):
        in_maps.append(dict(
            shared,
            x0t=np.ascontiguousarray(emb[b].T),
            cb=np.ascontiguousarray(
                np.broadcast_to(contrib[b][None, :], (128, S))),
            negc=np.ascontiguousarray(-contrib[b].reshape(QC, 128).T),
            rr=np.ascontiguousarray(reaches[b].reshape(KC, 128).T),
        ))
    return emb, in_maps


def kernel(src, reaches, emb_table, qw, kw, vw, ow):
    global LAST_RESULT
    if "nc" not in _NC_CACHE:
        _NC_CACHE["nc"] = _build(n_layers=NL, n_cores=B)
    nc = _NC_CACHE["nc"]
    emb, in_maps = _host_prep(src, reaches, emb_table, qw, kw, vw, ow)
    res = run_bass_kernel_spmd(nc, in_maps, core_ids=list(range(B)),
                               trace=TRACE)
    LAST_RESULT = res
    x = np.stack([r["xt"].T for r in res.results]).astype(np.float32)
    return emb, x



# revision 2
# speedup vs baseline: 1.1515x; 1.1515x over previous
"""TRN2 Bass kernel for the 4-layer encoder-with-reaches model
(nn_EncoderPreTre: B=8, S=512, D=1024, H=16 heads, NL=4 layers).

kernel(**inputs) takes the FULL inputs (src, reaches, emb_table,
qw/kw/vw/ow) and returns the full output tuple (emb, x) matching
reference.reference(). Distribution: data-parallel over the batch --
core b computes batch element b end to end (B == 8 == n_cores); the
embedding-row gather and per-batch contrib vectors are the host-side
sharding step.

v2 structure (per core, residual transposed x^T [1024, 512] in SBUF as
an f32r hi/lo pair):
  P1: q^T, k^T projections (3-term f32r hi/lo split: Wh@xh + Wh@xl +
      Wl@xh, fp32-accurate at ~3 PE cycles/row) -> fp32 tiles.
  P2: v^T projection, with the per-token contrib broadcast (cb) fused
      into the PSUM->SBUF copy: vcb = cb (.) v^T. PE transposes of vcb
      blocks + an ACT copy scaled by (reaches/contrib) produce
      vp[s, D] = reaches (.) v in fp16.
  P3 per head: scores fp32 (exactness needed at logit scale ~5.6e6) ->
      row-max -> E = exp(s-m) in fp16 with fp32 row-sum Z (accum_out) ->
      P = E*(-c/Z)*mask fp16 via one STT -> P transposed 128x128
      blockwise on the PE at 1 cycle/row (fp16) -> M2[dk,q] =
      sum_k vp[k,dk]*PT[k,q], a 1 cycle/row fp16 matmul (both P and v'
      carry ~11-bit mantissas; validated ~3.8e-3 final rel err).
  P4: a^T = M2-concat + vcb (the output of attention scaled by contrib,
      fused residual-free), split into an f32r hi/lo pair, and a single
      3-term-split wo projection updates the residual. This replaces the
      previous OV@x + wo@concat pair of projections (the v-term is
      carried through a^T instead), saving one full D x D x S projection
      and its 8MB/layer weight traffic.

Numerics: matmul rounding is amplified ~1000x across layers, so the
q/k/v/wo projections need fp32-equivalent accuracy (3-term f32r splits)
and the scores stay native fp32. Only the post-softmax P@v' path
tolerates ~11-bit operands (fp16), which is where the cheap matmuls go.
"""
import numpy as np

import concourse.tile as tile
from concourse import bacc, mybir
from concourse.bass_utils import run_bass_kernel_spmd

F32 = mybir.dt.float32
F32R = mybir.dt.float32r
BF16 = mybir.dt.bfloat16
FP16 = mybir.dt.float16
AX = mybir.AxisListType
OP = mybir.AluOpType
AF = mybir.ActivationFunctionType

B, S, D, H, DK, NL = 8, 512, 1024, 16, 64, 4
QC = S // 128
KC = S // 128
DC = D // 128


TRACE = False        # test harness sets True for neuron-profile capture
LAST_RESULT = None   # BassKernelResults of the last kernel() call
_NC_CACHE = {}


def _build(n_layers=NL, n_cores=8):
    nc = bacc.Bacc("TRN2", target_bir_lowering=False, debug=False,
                   num_devices=n_cores)
    d_x0 = nc.dram_tensor("x0t", [D, S], F32, kind="ExternalInput").ap()
    dw = {}
    for nm in ["wq", "wk", "wv", "wo"]:
        dw[nm + "h"] = nc.dram_tensor(nm + "h", [NL, D, D], F32R,
                                      kind="ExternalInput").ap()
        dw[nm + "l"] = nc.dram_tensor(nm + "l", [NL, D, D], F32R,
                                      kind="ExternalInput").ap()
    d_cb = nc.dram_tensor("cb", [128, S], F32, kind="ExternalInput").ap()
    d_negc = nc.dram_tensor("negc", [128, QC], F32, kind="ExternalInput").ap()
    d_rrc = nc.dram_tensor("rrc", [128, KC], F32, kind="ExternalInput").ap()
    d_mask = nc.dram_tensor("maskq", [QC, 128, S], BF16, kind="ExternalInput").ap()
    d_id32 = nc.dram_tensor("ident", [128, 128], F32, kind="ExternalInput").ap()
    d_id16 = nc.dram_tensor("ident16", [128, 128], FP16, kind="ExternalInput").ap()
    d_out = nc.dram_tensor("xt", [D, S], F32, kind="ExternalOutput").ap()

    with tile.TileContext(nc) as tc:
        _emit(nc, tc, n_layers, d_x0, dw,
              d_cb, d_negc, d_rrc, d_mask, d_id32, d_id16, d_out)
    nc.compile()
    return nc


def _emit(nc, tc, n_layers, d_x0, dw, d_cb, d_negc, d_rrc, d_mask,
          d_id32, d_id16, d_out):
    ctx_pools = []

    def pool(name, bufs, space="SBUF"):
        p = tc.tile_pool(name=name, bufs=bufs, space=space)
        ctx_pools.append(p)
        return p.__enter__()

    const = pool("const", 1)
    xpool = pool("x", 1)
    actp = pool("act", 1)
    wpool = pool("w", 1)          # 16 tags (hi/lo per ki); ki-granular overlap
    epool = pool("E", 2)
    ppool = pool("P", 4)
    ptpool = pool("PT", 4)
    small = pool("small", 3)
    tmp8 = pool("tmp8", 1)
    psA = pool("psA", 3, "PSUM")
    psT = pool("psT", 2, "PSUM")
    psM = pool("psM", 2, "PSUM")

    cb = const.tile([128, S], F32)
    nc.sync.dma_start(cb[:], d_cb)
    negc = const.tile([128, QC], F32)
    nc.sync.dma_start(negc[:], d_negc)
    rrc = const.tile([128, KC], F32)
    nc.sync.dma_start(rrc[:], d_rrc)
    id32 = const.tile([128, 128], F32)
    nc.sync.dma_start(id32[:], d_id32)
    id16 = const.tile([128, 128], FP16)
    nc.sync.dma_start(id16[:], d_id16)
    masks = []
    for t in range(QC):
        mt = const.tile([128, S], BF16, tag=f"mask{t}", name=f"mask{t}")
        nc.sync.dma_start(mt[:], d_mask[t])
        masks.append(mt)

    # residual pair: x ~= xhi + xlo (f32r each)
    xhi, xlo = [], []
    for c in range(DC):
        xh = xpool.tile([128, S], F32R, tag=f"xh{c}", name=f"xh{c}")
        xl = xpool.tile([128, S], F32R, tag=f"xl{c}", name=f"xl{c}")
        xhi.append(xh)
        xlo.append(xl)
    for c in range(DC):
        xf = tmp8.tile([128, S], F32, tag="xa", name=f"x0f{c}", bufs=1)
        nc.sync.dma_start(xf[:], d_x0[c * 128:(c + 1) * 128, :])
        nc.vector.tensor_copy(xhi[c][:], xf[:])
        nc.vector.tensor_tensor(xlo[c][:], xf[:], xhi[c][:], op=OP.subtract)

    for l in range(n_layers):
        def load_split(nm):
            his, los = [], []
            for ki in range(DC):
                wh = wpool.tile([128, D], F32R, tag=f"w{ki}h", name=f"{nm}h{ki}_{l}")
                nc.sync.dma_start(wh[:], dw[nm + "h"][l, ki * 128:(ki + 1) * 128, :])
                wl = wpool.tile([128, D], F32R, tag=f"w{ki}l", name=f"{nm}l{ki}_{l}")
                nc.sync.dma_start(wl[:], dw[nm + "l"][l, ki * 128:(ki + 1) * 128, :])
                his.append(wh)
                los.append(wl)
            return his, los

        def proj_psums(his, los, outtag, rhs_hi, rhs_lo):
            """3-term-split projection; yields (c, psum_tile) pairs."""
            for c in range(DC):
                p = psA.tile([128, S], F32, tag="psA", name=f"pp{outtag}{c}_{l}")
                n_mm = 3 * DC
                i_mm = 0
                sl = slice(c * 128, (c + 1) * 128)
                for ki in range(DC):
                    for lhsT, rhs in ((his[ki][:, sl], rhs_hi[ki][:]),
                                      (his[ki][:, sl], rhs_lo[ki][:]),
                                      (los[ki][:, sl], rhs_hi[ki][:])):
                        nc.tensor.matmul(
                            p[:], lhsT, rhs, start=(i_mm == 0),
                            stop=(i_mm == n_mm - 1), skip_group_check=True)
                        i_mm += 1
                yield c, p

        # ---- q, k projections (fp32 tiles for exact fp32 scores) ----
        qh_w, ql_w = load_split("wq")
        qt = []
        for c, p in proj_psums(qh_w, ql_w, "qt", xhi, xlo):
            o = actp.tile([128, S], F32, tag=f"qt{c}", name=f"qt{c}_{l}")
            nc.vector.tensor_copy(o[:], p[:])
            qt.append(o)
        kh_w, kl_w = load_split("wk")
        kt = []
        for c, p in proj_psums(kh_w, kl_w, "kt", xhi, xlo):
            o = actp.tile([128, S], F32, tag=f"kt{c}", name=f"kt{c}_{l}")
            nc.vector.tensor_copy(o[:], p[:])
            kt.append(o)

        # ---- v^T projection with cb fused: vcb = cb (.) v^T ----
        vh_w, vl_w = load_split("wv")
        vcb = []
        for c, p in proj_psums(vh_w, vl_w, "vt", xhi, xlo):
            o = actp.tile([128, S], F32, tag=f"vt{c}", name=f"vt{c}_{l}")
            nc.vector.tensor_tensor(o[:], p[:], cb[:], op=OP.mult)
            vcb.append(o)

        # wo weights early: DMA overlaps the whole attention phase
        woh_w, wol_w = load_split("wo")

        # ---- vp[s, D] = reaches (.) v in fp16, via PE transposes ----
        # vp = transpose(vcb) * (reaches/contrib) = reaches (.) v
        vp = [actp.tile([128, D], FP16, tag=f"vp{sc}", name=f"vp{sc}_{l}")
              for sc in range(KC)]
        for sc in range(KC):
            for half in range(2):
                ps = psA.tile([128, S], F32, tag="psA", name=f"pv{sc}{half}_{l}")
                for j in range(4):
                    c = half * 4 + j
                    nc.tensor.matmul(
                        ps[:, j * 128:(j + 1) * 128],
                        vcb[c][:, sc * 128:(sc + 1) * 128], id32[:],
                        is_transpose=True, start=(j == 0), stop=(j == 3),
                        skip_group_check=True)
                nc.scalar.activation(
                    vp[sc][:, half * 512:(half + 1) * 512], ps[:], AF.Copy,
                    scale=rrc[:, sc:sc + 1])

        # a^T accumulator: per head, af += m2; vcb term added per head
        af = [actp.tile([128, S], F32, tag=f"af{c}", name=f"af{c}_{l}")
              for c in range(DC)]

        # ---- attention heads ----
        for h in range(H):
            hp = h // 2
            hb = (h % 2) * 64
            qsl = qt[hp][hb:hb + 64, :]
            ksl = kt[hp][hb:hb + 64, :]

            negm = small.tile([128, QC], F32, tag="negm", name=f"negm{h}_{l}")
            zst = small.tile([128, QC], F32, tag="zst", name=f"zst{h}_{l}")
            sc_t = small.tile([128, QC], F32, tag="scl", name=f"scl{h}_{l}")
            Ps = []
            for t in range(QC):
                ps = psA.tile([128, S], F32, tag="psA", name=f"sc{h}{t}_{l}")
                nc.tensor.matmul(ps[:], qsl[:, t * 128:(t + 1) * 128], ksl,
                                 start=True, stop=True)
                nc.vector.tensor_reduce(
                    negm[:, t:t + 1], ps[:], axis=AX.X, op=OP.max, negate=True)
                e = epool.tile([128, S], FP16, tag="E", name=f"e{h}{t}_{l}")
                nc.scalar.activation(e[:], ps[:], AF.Exp,
                                     bias=negm[:, t:t + 1], scale=1.0,
                                     accum_out=zst[:, t:t + 1])
                nc.vector.reciprocal(sc_t[:, t:t + 1], zst[:, t:t + 1])
                nc.vector.tensor_tensor(
                    sc_t[:, t:t + 1], sc_t[:, t:t + 1], negc[:, t:t + 1],
                    op=OP.mult)
                p = ppool.tile([128, S], FP16, tag="P", name=f"p{h}{t}_{l}")
                nc.vector.scalar_tensor_tensor(
                    p[:], e[:], sc_t[:, t:t + 1], masks[t][:],
                    op0=OP.mult, op1=OP.mult)
                Ps.append(p)

            PTs = []
            for kc in range(KC):
                tp = psT.tile([128, S], FP16, tag="psT", name=f"tp{h}{kc}_{l}")
                for t in range(QC):
                    nc.tensor.matmul(
                        tp[:, t * 128:(t + 1) * 128],
                        Ps[t][:, kc * 128:(kc + 1) * 128], id16[:],
                        is_transpose=True, start=(t == 0), stop=(t == QC - 1),
                        skip_group_check=True)
                pt_sb = ptpool.tile([128, S], FP16, tag="PT", name=f"pt{h}{kc}_{l}")
                if kc % 2 == 0:
                    nc.vector.tensor_copy(pt_sb[:], tp[:])
                else:
                    nc.scalar.copy(pt_sb[:], tp[:])
                PTs.append(pt_sb)

            m2 = psM.tile([128, S], F32, tag="psM", name=f"m2{h}_{l}")
            off = hb
            for kc in range(KC):
                nc.tensor.matmul(
                    m2[off:off + 64, :], vp[kc][:, h * 64:h * 64 + 64],
                    PTs[kc][:], start=(kc == 0), stop=(kc == KC - 1))
            # a^T[dk-rows of this head] = m2 + cb (.) v^T
            nc.vector.tensor_tensor(
                af[hp][hb:hb + 64, :], m2[off:off + 64, :],
                vcb[hp][hb:hb + 64, :], op=OP.add)

        # ---- split a^T into f32r hi/lo (reusing q/k tags) ----
        ahs, als = [], []
        for c in range(DC):
            ah = actp.tile([128, S], F32R, tag=f"qt{c}", name=f"ah{c}_{l}")
            nc.vector.tensor_copy(ah[:], af[c][:])   # f32r copy rounds
            al = actp.tile([128, S], F32R, tag=f"kt{c}", name=f"al{c}_{l}")
            nc.gpsimd.tensor_tensor(al[:], af[c][:], ah[:], op=OP.subtract)
            ahs.append(ah)
            als.append(al)

        # ---- single wo projection (3-term split) + residual update ----
        for c in range(DC):
            pow_ = psA.tile([128, S], F32, tag="psA", name=f"pow{c}_{l}")
            n_mm = 3 * DC
            i_mm = 0
            sl = slice(c * 128, (c + 1) * 128)
            for ki in range(DC):
                for lhsT, rhs in ((woh_w[ki][:, sl], ahs[ki][:]),
                                  (woh_w[ki][:, sl], als[ki][:]),
                                  (wol_w[ki][:, sl], ahs[ki][:])):
                    nc.tensor.matmul(pow_[:], lhsT, rhs, start=(i_mm == 0),
                                     stop=(i_mm == n_mm - 1),
                                     skip_group_check=True)
                    i_mm += 1
            xa = tmp8.tile([128, S], F32, tag="xa", name=f"xa{c}_{l}", bufs=1)
            nc.gpsimd.tensor_tensor(xa[:], xhi[c][:], xlo[c][:], op=OP.add)
            xb = tmp8.tile([128, S], F32, tag="xb", name=f"xb{c}_{l}", bufs=1)
            nc.vector.tensor_tensor(xb[:], xa[:], pow_[:], op=OP.add)
            if l == n_layers - 1:
                nc.sync.dma_start(d_out[sl, :], xb[:])
            else:
                nc.vector.tensor_copy(xhi[c][:], xb[:])   # f32r copy rounds
                nc.gpsimd.tensor_tensor(xlo[c][:], xb[:], xhi[c][:],
                                        op=OP.subtract)

    for p in reversed(ctx_pools):
        p.__exit__(None, None, None)


# ---------------- host side ----------------

def _t13(a):
    u = np.ascontiguousarray(np.asarray(a, np.float32)).view(np.uint32)
    r = (u + np.uint32(1 << 12)) & ~np.uint32((1 << 13) - 1)
    return r.view(np.float32)


def _host_prep(src, reaches, emb_table, qw, kw, vw, ow):
    src = np.asarray(src)
    reaches = np.asarray(reaches, dtype=np.float32)
    emb_table = np.asarray(emb_table, dtype=np.float32)
    emb = emb_table[src]
    rs = reaches.sum(-1, keepdims=True)
    contrib = ((rs - reaches) / (rs + 1e-9) * (1.0 - reaches) * 100.0
               ).astype(np.float32)

    qw = np.asarray(qw, np.float32); kw = np.asarray(kw, np.float32)
    vw = np.asarray(vw, np.float32); ow = np.asarray(ow, np.float32)
    wq = np.ascontiguousarray(np.transpose(qw, (0, 2, 1)) * 0.125)
    wk = np.ascontiguousarray(np.transpose(kw, (0, 2, 1)))
    wv = np.ascontiguousarray(np.transpose(vw, (0, 2, 1)))
    wo = np.ascontiguousarray(np.transpose(ow, (0, 2, 1)))

    def split(w):
        h = _t13(w)
        lo = _t13(w - h)
        return np.ascontiguousarray(h), np.ascontiguousarray(lo)

    wqh, wql = split(wq)
    wkh, wkl = split(wk)
    wvh, wvl = split(wv)
    woh, wol = split(wo)

    import ml_dtypes
    maskq = np.ones((QC, 128, S), ml_dtypes.bfloat16)
    idx = np.arange(128)
    diagval = np.float32(1.0) - np.float32(0.999999)
    for t in range(QC):
        maskq[t, idx, t * 128 + idx] = ml_dtypes.bfloat16(diagval)
    ident = np.eye(128, dtype=np.float32)
    ident16 = np.eye(128, dtype=np.float16)

    shared = dict(wqh=wqh, wql=wql, wkh=wkh, wkl=wkl, wvh=wvh, wvl=wvl,
                  woh=woh, wol=wol, maskq=maskq, ident=ident, ident16=ident16)
    rr_over_c = (reaches / contrib).astype(np.float32)
    in_maps = []
    for b in range(B):
        in_maps.append(dict(
            shared,
            x0t=np.ascontiguousarray(emb[b].T),
            cb=np.ascontiguousarray(
                np.broadcast_to(contrib[b][None, :], (128, S))),
            negc=np.ascontiguousarray(-contrib[b].reshape(QC, 128).T),
            rrc=np.ascontiguousarray(rr_over_c[b].reshape(KC, 128).T),
        ))
    return emb, in_maps


def kernel(src, reaches, emb_table, qw, kw, vw, ow):
    global LAST_RESULT
    if "nc" not in _NC_CACHE:
        _NC_CACHE["nc"] = _build(n_layers=NL, n_cores=B)
    nc = _NC_CACHE["nc"]
    emb, in_maps = _host_prep(src, reaches, emb_table, qw, kw, vw, ow)
    res = run_bass_kernel_spmd(nc, in_maps, core_ids=list(range(B)),
                               trace=TRACE)
    LAST_RESULT = res
    x = np.stack([r["xt"].T for r in res.results]).astype(np.float32)
    return emb, x


# revision 7
# speedup vs baseline: 1.1795x; 1.0243x over previous
"""TRN2 Bass kernel for the 4-layer encoder-with-reaches model
(nn_EncoderPreTre: B=8, S=512, D=1024, H=16 heads, NL=4 layers).

kernel(**inputs) takes the FULL inputs (src, reaches, emb_table,
qw/kw/vw/ow) and returns the full output tuple (emb, x) matching
reference.reference(). Distribution: data-parallel over the batch --
core b computes batch element b end to end (B == 8 == n_cores); the
embedding-row gather and per-batch contrib vectors are the host-side
sharding step.

v3 structure (per core, residual transposed x^T [1024, 512] in SBUF):
  Every precision-critical matmul runs as a 3-term fp16 hi/lo-pair
  split (Ah@Bh + Ah@Bl + Al@Bh) at 1 PE cycle/row: a fp16 pair carries
  ~22 mantissa bits, and term products accumulate in fp32 PSUM, giving
  fp32-class accuracy at 3 cycles/row (vs 4 for native fp32).
  Weights are pre-scaled x64 on the host so their lo-halves clear the
  fp16 subnormal floor; the scale is removed with exact power-of-2
  multiplies at the PSUM->SBUF split points (q also folds in the 1/8
  softmax scale: unscale 1/512).

  P1: q^T, k^T projections -> fp16 hi/lo pairs (q at 1/512, k at 1/64).
  P2: v^T projection with contrib/64 fused into the PSUM copy
      (vcb = cb (.) v^T, fp32); PE transposes + an ACT copy scaled by
      reaches/contrib produce vp[s, D] = reaches (.) v in fp16.
  P3 heads, software-pipelined with a 1-head skew (head h's softmax
      chain on DVE/ACT overlaps head h-1's transposes/M2 on the PE):
      scores = 3-term fp16 matmul -> row-max -> E = exp(s-m) fp16 with
      fp32 row-sum Z -> P = E*(-c/Z)*mask fp16 -> PE transpose (1
      cyc/row) -> M2[dk,q] = sum_k vp[k,dk]*PT[k,q] fp16.
  P4: a^T = M2-concat + vcb, split into an fp16 pair at 1/64, then a
      single 3-term wo projection (wo x64) updates the residual. The
      v-term rides through a^T, eliminating the OV@x projection.

Numerics: validated host-side vs the fp32 jax reference at ~3.8e-3 max
rel err (gate 2e-2), including exact fp16 rounding/subnormal behavior.
"""
import numpy as np

import concourse.tile as tile
from concourse import bacc, mybir
from concourse.bass_utils import run_bass_kernel_spmd

F32 = mybir.dt.float32
F32R = mybir.dt.float32r
BF16 = mybir.dt.bfloat16
FP16 = mybir.dt.float16
AX = mybir.AxisListType
OP = mybir.AluOpType
AF = mybir.ActivationFunctionType

B, S, D, H, DK, NL = 8, 512, 1024, 16, 64, 4
QC = S // 128
KC = S // 128
DC = D // 128


TRACE = False        # test harness sets True for neuron-profile capture
LAST_RESULT = None   # BassKernelResults of the last kernel() call
_NC_CACHE = {}


def _build(n_layers=NL, n_cores=8):
    nc = bacc.Bacc("TRN2", target_bir_lowering=False, debug=False,
                   num_devices=n_cores)
    d_x0 = nc.dram_tensor("x0t", [D, S], F32, kind="ExternalInput").ap()
    dw = {}
    for nm in ["wq", "wk", "wv", "wo"]:
        dw[nm + "h"] = nc.dram_tensor(nm + "h", [NL, D, D], FP16,
                                      kind="ExternalInput").ap()
        dw[nm + "l"] = nc.dram_tensor(nm + "l", [NL, D, D], FP16,
                                      kind="ExternalInput").ap()
    d_cb = nc.dram_tensor("cb", [128, S], F32, kind="ExternalInput").ap()
    d_negc = nc.dram_tensor("negc", [128, QC], F32, kind="ExternalInput").ap()
    d_rrc = nc.dram_tensor("rrc", [128, KC], F32, kind="ExternalInput").ap()
    d_mask = nc.dram_tensor("maskq", [QC, 128, S], FP16, kind="ExternalInput").ap()
    d_id32 = nc.dram_tensor("ident", [128, 128], F32, kind="ExternalInput").ap()
    d_id16 = nc.dram_tensor("ident16", [128, 128], FP16, kind="ExternalInput").ap()
    d_out = nc.dram_tensor("xt", [D, S], F32, kind="ExternalOutput").ap()

    with tile.TileContext(nc) as tc:
        _emit(nc, tc, n_layers, d_x0, dw,
              d_cb, d_negc, d_rrc, d_mask, d_id32, d_id16, d_out)
    nc.compile()
    return nc


def _emit(nc, tc, n_layers, d_x0, dw, d_cb, d_negc, d_rrc, d_mask,
          d_id32, d_id16, d_out):
    ctx_pools = []

    def pool(name, bufs, space="SBUF"):
        p = tc.tile_pool(name=name, bufs=bufs, space=space)
        ctx_pools.append(p)
        return p.__enter__()

    const = pool("const", 1)
    xpool = pool("x", 1)
    actp = pool("act", 1)
    wpool = pool("w", 2)          # 16 tags, double-buffered for prefetch
    epool = pool("E", 2)
    ppool = pool("P", 2)          # per-t tags; 2 bufs spans the head skew
    ptpool = pool("PT", 4)
    small = pool("small", 3)
    tmp8 = pool("tmp8", 2)
    psA = pool("psA", 4, "PSUM")
    psT = pool("psT", 2, "PSUM")
    psM = pool("psM", 2, "PSUM")

    cb = const.tile([128, S], F32)
    nc.sync.dma_start(cb[:], d_cb)
    negc = const.tile([128, QC], F32)
    nc.sync.dma_start(negc[:], d_negc)
    rrc = const.tile([128, KC], F32)
    nc.sync.dma_start(rrc[:], d_rrc)
    id32 = const.tile([128, 128], F32)
    nc.sync.dma_start(id32[:], d_id32)
    id16 = const.tile([128, 128], FP16)
    nc.sync.dma_start(id16[:], d_id16)
    masks = []
    for t in range(QC):
        mt = const.tile([128, S], FP16, tag=f"mask{t}", name=f"mask{t}")
        nc.sync.dma_start(mt[:], d_mask[t])
        masks.append(mt)

    # residual pair: x ~= xhi + xlo (fp16 each; ~22 mantissa bits total)
    xhi, xlo = [], []
    for c in range(DC):
        xh = xpool.tile([128, S], FP16, tag=f"xh{c}", name=f"xh{c}")
        xl = xpool.tile([128, S], FP16, tag=f"xl{c}", name=f"xl{c}")
        xhi.append(xh)
        xlo.append(xl)
    for c in range(DC):
        xf = tmp8.tile([128, S], F32, tag="xa", name=f"x0f{c}", bufs=2)
        nc.sync.dma_start(xf[:], d_x0[c * 128:(c + 1) * 128, :])
        nc.vector.tensor_copy(xhi[c][:], xf[:])
        nc.vector.tensor_tensor(xlo[c][:], xf[:], xhi[c][:], op=OP.subtract)

    for l in range(n_layers):
        def load_split(nm):
            his, los = [], []
            for ki in range(DC):
                wh = wpool.tile([128, D], FP16, tag=f"w{ki}h", name=f"{nm}h{ki}_{l}")
                nc.sync.dma_start(wh[:], dw[nm + "h"][l, ki * 128:(ki + 1) * 128, :])
                wl = wpool.tile([128, D], FP16, tag=f"w{ki}l", name=f"{nm}l{ki}_{l}")
                nc.sync.dma_start(wl[:], dw[nm + "l"][l, ki * 128:(ki + 1) * 128, :])
                his.append(wh)
                los.append(wl)
            return his, los

        def proj_psums(his, los, outtag, rhs_hi, rhs_lo):
            """3-term-split projection; yields (c, psum_tile) pairs."""
            for c in range(DC):
                p = psA.tile([128, S], F32, tag="psA", name=f"pp{outtag}{c}_{l}")
                n_mm = 3 * DC
                i_mm = 0
                sl = slice(c * 128, (c + 1) * 128)
                for ki in range(DC):
                    for lhsT, rhs in ((his[ki][:, sl], rhs_hi[ki][:]),
                                      (his[ki][:, sl], rhs_lo[ki][:]),
                                      (los[ki][:, sl], rhs_hi[ki][:])):
                        nc.tensor.matmul(
                            p[:], lhsT, rhs, start=(i_mm == 0),
                            stop=(i_mm == n_mm - 1), skip_group_check=True)
                        i_mm += 1
                yield c, p

        def split16(p, hi_t, lo_t, unscale):
            """PSUM -> fp16 hi/lo pair with exact power-of-2 unscale."""
            nc.vector.tensor_scalar(hi_t[:], p[:], unscale, None, op0=OP.mult)
            nc.vector.scalar_tensor_tensor(
                lo_t[:], p[:], unscale, hi_t[:], op0=OP.mult, op1=OP.subtract)

        # ---- q, k projections -> fp16 pairs (q carries the 1/8 scale) ----
        whq, wlq = load_split("wq")
        qhi, qlo = [], []
        for c, p in proj_psums(whq, wlq, "qt", xhi, xlo):
            qh = actp.tile([128, S], FP16, tag=f"qh{c}", name=f"qh{c}_{l}")
            ql = actp.tile([128, S], FP16, tag=f"ql{c}", name=f"ql{c}_{l}")
            split16(p, qh, ql, 1.0 / 512)
            qhi.append(qh)
            qlo.append(ql)
        whk, wlk = load_split("wk")
        khi, klo = [], []
        for c, p in proj_psums(whk, wlk, "kt", xhi, xlo):
            kh = actp.tile([128, S], FP16, tag=f"kh{c}", name=f"kh{c}_{l}")
            kl = actp.tile([128, S], FP16, tag=f"kl{c}", name=f"kl{c}_{l}")
            split16(p, kh, kl, 1.0 / 64)
            khi.append(kh)
            klo.append(kl)

        # ---- v^T projection with cb fused: vcb = (contrib/64) (.) v64^T ----
        whv, wlv = load_split("wv")
        vcb = []
        for c, p in proj_psums(whv, wlv, "vt", xhi, xlo):
            o = actp.tile([128, S], F32, tag=f"vt{c}", name=f"vt{c}_{l}")
            nc.vector.tensor_tensor(o[:], p[:], cb[:], op=OP.mult)
            vcb.append(o)

        # wo weights early: DMA overlaps the whole attention phase
        woh_w, wol_w = load_split("wo")

        # ---- vp[s, D] = reaches (.) v in fp16, via PE transposes ----
        vp = [actp.tile([128, D], FP16, tag=f"vp{sc}", name=f"vp{sc}_{l}")
              for sc in range(KC)]
        for sc in range(KC):
            for half in range(2):
                ps = psA.tile([128, S], F32, tag="psA", name=f"pv{sc}{half}_{l}")
                for j in range(4):
                    c = half * 4 + j
                    nc.tensor.matmul(
                        ps[:, j * 128:(j + 1) * 128],
                        vcb[c][:, sc * 128:(sc + 1) * 128], id32[:],
                        is_transpose=True, start=(j == 0), stop=(j == 3),
                        skip_group_check=True)
                nc.scalar.activation(
                    vp[sc][:, half * 512:(half + 1) * 512], ps[:], AF.Copy,
                    scale=rrc[:, sc:sc + 1])

        # a^T accumulator: per head, af[head rows] = m2 + vcb
        af = [actp.tile([128, S], F32, tag=f"af{c}", name=f"af{c}_{l}")
              for c in range(DC)]

        # ---- attention heads, software-pipelined with a 1-head skew ----
        ahs, als = [None] * DC, [None] * DC

        def emit_scores(h):
            hp = h // 2
            hb = (h % 2) * 64
            negm = small.tile([128, QC], F32, tag="negm", name=f"negm{h}_{l}")
            zst = small.tile([128, QC], F32, tag="zst", name=f"zst{h}_{l}")
            sc_t = small.tile([128, QC], F32, tag="scl", name=f"scl{h}_{l}")
            Ps = []
            for t in range(QC):
                ps = psA.tile([128, S], F32, tag="psA", name=f"sc{h}{t}_{l}")
                tsl = slice(t * 128, (t + 1) * 128)
                hsl = slice(hb, hb + 64)
                i_mm = 0
                for lhsT, rhs in ((qhi[hp][hsl, tsl], khi[hp][hsl, :]),
                                  (qhi[hp][hsl, tsl], klo[hp][hsl, :]),
                                  (qlo[hp][hsl, tsl], khi[hp][hsl, :])):
                    nc.tensor.matmul(ps[:], lhsT, rhs, start=(i_mm == 0),
                                     stop=(i_mm == 2), skip_group_check=True)
                    i_mm += 1
                nc.vector.tensor_reduce(
                    negm[:, t:t + 1], ps[:], axis=AX.X, op=OP.max, negate=True)
                e = epool.tile([128, S], FP16, tag="E", name=f"e{h}{t}_{l}")
                nc.scalar.activation(e[:], ps[:], AF.Exp,
                                     bias=negm[:, t:t + 1], scale=1.0,
                                     accum_out=zst[:, t:t + 1])
                nc.vector.reciprocal(sc_t[:, t:t + 1], zst[:, t:t + 1])
                nc.vector.tensor_tensor(
                    sc_t[:, t:t + 1], sc_t[:, t:t + 1], negc[:, t:t + 1],
                    op=OP.mult)
                p = ppool.tile([128, S], FP16, tag=f"P{t}", name=f"p{h}{t}_{l}")
                nc.vector.scalar_tensor_tensor(
                    p[:], e[:], sc_t[:, t:t + 1], masks[t][:],
                    op0=OP.mult, op1=OP.mult)
                Ps.append(p)
            return Ps

        def emit_tail(h, Ps):
            hp = h // 2
            hb = (h % 2) * 64
            PTs = []
            for kc in range(KC):
                tp = psT.tile([128, S], FP16, tag="psT", name=f"tp{h}{kc}_{l}")
                for t in range(QC):
                    nc.tensor.matmul(
                        tp[:, t * 128:(t + 1) * 128],
                        Ps[t][:, kc * 128:(kc + 1) * 128], id16[:],
                        is_transpose=True, start=(t == 0), stop=(t == QC - 1),
                        skip_group_check=True)
                pt_sb = ptpool.tile([128, S], FP16, tag="PT", name=f"pt{h}{kc}_{l}")
                if kc % 2 == 0:
                    nc.vector.tensor_copy(pt_sb[:], tp[:])
                else:
                    nc.scalar.copy(pt_sb[:], tp[:])
                PTs.append(pt_sb)

            m2 = psM.tile([128, S], F32, tag="psM", name=f"m2{h}_{l}")
            off = hb
            for kc in range(KC):
                nc.tensor.matmul(
                    m2[off:off + 64, :], vp[kc][:, h * 64:h * 64 + 64],
                    PTs[kc][:], start=(kc == 0), stop=(kc == KC - 1))
            # a^T[dk-rows of this head] = m2 + cb (.) v^T
            nc.vector.tensor_tensor(
                af[hp][hb:hb + 64, :], m2[off:off + 64, :],
                vcb[hp][hb:hb + 64, :], op=OP.add)
            if h % 2 == 1:
                # both head-halves of af[hp] done: split into an fp16 pair
                # at 1/64 now so the wo projection never waits
                ah = actp.tile([128, S], FP16, tag=f"qh{hp}", name=f"ah{hp}_{l}")
                nc.vector.tensor_scalar(ah[:], af[hp][:], 1.0 / 64, None,
                                        op0=OP.mult)
                al = actp.tile([128, S], FP16, tag=f"kh{hp}", name=f"al{hp}_{l}")
                nc.vector.scalar_tensor_tensor(
                    al[:], af[hp][:], 1.0 / 64, ah[:],
                    op0=OP.mult, op1=OP.subtract)
                ahs[hp] = ah
                als[hp] = al

        prev = None
        for h in range(H):
            Ps = emit_scores(h)
            if prev is not None:
                emit_tail(prev[0], prev[1])
            prev = (h, Ps)
        emit_tail(prev[0], prev[1])

        # ---- single wo projection (3-term split) + residual update ----
        for c in range(DC):
            pow_ = psA.tile([128, S], F32, tag="psA", name=f"pow{c}_{l}")
            n_mm = 3 * DC
            i_mm = 0
            sl = slice(c * 128, (c + 1) * 128)
            for ki in range(DC):
                for lhsT, rhs in ((woh_w[ki][:, sl], ahs[ki][:]),
                                  (woh_w[ki][:, sl], als[ki][:]),
                                  (wol_w[ki][:, sl], ahs[ki][:])):
                    nc.tensor.matmul(pow_[:], lhsT, rhs, start=(i_mm == 0),
                                     stop=(i_mm == n_mm - 1),
                                     skip_group_check=True)
                    i_mm += 1
            xa = tmp8.tile([128, S], F32, tag="xa", name=f"xa{c}_{l}", bufs=2)
            nc.gpsimd.tensor_tensor(xa[:], xhi[c][:], xlo[c][:], op=OP.add)
            xb = tmp8.tile([128, S], F32, tag="xb", name=f"xb{c}_{l}", bufs=2)
            nc.vector.tensor_tensor(xb[:], xa[:], pow_[:], op=OP.add)
            if l == n_layers - 1:
                nc.sync.dma_start(d_out[sl, :], xb[:])
            else:
                nc.vector.tensor_copy(xhi[c][:], xb[:])   # fp16 copy rounds
                nc.gpsimd.tensor_tensor(xlo[c][:], xb[:], xhi[c][:],
                                        op=OP.subtract)

    for p in reversed(ctx_pools):
        p.__exit__(None, None, None)


# ---------------- host side ----------------

def _halfpair(w):
    """fp16 hi/lo pair of w (already pre-scaled)."""
    h = w.astype(np.float16)
    lo = (w - h.astype(np.float32)).astype(np.float16)
    return np.ascontiguousarray(h), np.ascontiguousarray(lo)


def _host_prep(src, reaches, emb_table, qw, kw, vw, ow):
    src = np.asarray(src)
    reaches = np.asarray(reaches, dtype=np.float32)
    emb_table = np.asarray(emb_table, dtype=np.float32)
    emb = emb_table[src]
    rs = reaches.sum(-1, keepdims=True)
    contrib = ((rs - reaches) / (rs + 1e-9) * (1.0 - reaches) * 100.0
               ).astype(np.float32)

    qw = np.asarray(qw, np.float32); kw = np.asarray(kw, np.float32)
    vw = np.asarray(vw, np.float32); ow = np.asarray(ow, np.float32)
    # x64 so the fp16 lo-halves clear the subnormal floor; unscaled at
    # the PSUM split points (q additionally folds the 1/8 softmax scale)
    wq = np.ascontiguousarray(np.transpose(qw, (0, 2, 1)) * 64.0)
    wk = np.ascontiguousarray(np.transpose(kw, (0, 2, 1)) * 64.0)
    wv = np.ascontiguousarray(np.transpose(vw, (0, 2, 1)) * 64.0)
    wo = np.ascontiguousarray(np.transpose(ow, (0, 2, 1)) * 64.0)

    wqh, wql = _halfpair(wq)
    wkh, wkl = _halfpair(wk)
    wvh, wvl = _halfpair(wv)
    woh, wol = _halfpair(wo)

    maskq = np.ones((QC, 128, S), np.float16)
    idx = np.arange(128)
    diagval = np.float32(1.0) - np.float32(0.999999)
    for t in range(QC):
        maskq[t, idx, t * 128 + idx] = np.float16(diagval)
    ident = np.eye(128, dtype=np.float32)
    ident16 = np.eye(128, dtype=np.float16)

    shared = dict(wqh=wqh, wql=wql, wkh=wkh, wkl=wkl, wvh=wvh, wvl=wvl,
                  woh=woh, wol=wol, maskq=maskq, ident=ident, ident16=ident16)
    rr_over_c = (reaches / contrib).astype(np.float32)
    in_maps = []
    for b in range(B):
        in_maps.append(dict(
            shared,
            x0t=np.ascontiguousarray(emb[b].T),
            cb=np.ascontiguousarray(
                np.broadcast_to(contrib[b][None, :] / 64.0, (128, S))),
            negc=np.ascontiguousarray(-contrib[b].reshape(QC, 128).T),
            rrc=np.ascontiguousarray(rr_over_c[b].reshape(KC, 128).T),
        ))
    return emb, in_maps


def kernel(src, reaches, emb_table, qw, kw, vw, ow):
    global LAST_RESULT
    if "nc" not in _NC_CACHE:
        _NC_CACHE["nc"] = _build(n_layers=NL, n_cores=B)
    nc = _NC_CACHE["nc"]
    emb, in_maps = _host_prep(src, reaches, emb_table, qw, kw, vw, ow)
    res = run_bass_kernel_spmd(nc, in_maps, core_ids=list(range(B)),
                               trace=TRACE)
    LAST_RESULT = res
    x = np.stack([r["xt"].T for r in res.results]).astype(np.float32)
    return emb, x
